# revision 6
# baseline (speedup 1.0000x reference)
"""Trainium2 kernel for nn_InterpolatorMaskArgs (embedding_lookup, memory regime).

reference computes:  ind = floor((x[0]-X0)/DX);  res = sum(roll(mask, ind) * yOrig)
i.e. a full O(N) dot product between yOrig and the rolled mask, with an
out-of-range guard on x.

Strategy (matches the sharding hint):
  - 1-D shard yOrig along N across the 8 cores (contiguous 2M-element shards).
  - The roll is resolved at shard time: core c receives the slice of the
    rolled mask aligned with its yOrig shard, i.e. mask[(c*S - ind) mod N ...]
    (mod-N wraparound == the halo exchange, done while scattering inputs).
  - Reduced-precision HBM traffic (tolerance is 2e-2; measured end-to-end rel
    err ~1e-3): y is sent as fp16 (4 MiB/core) and the mask as fp8e4
    (2 MiB/core; 0.5 is exact in e4m3).  6 MiB/core @ ~358 GB/s.
  - Two DMA queues run concurrently so per-transfer completion overheads
    overlap: y chunks on the sync HWDGE ring; mask chunks on the gpsimd
    SWDGE ring, which casts fp8 -> fp16 inline during the DMA (HBM reads
    1 B/elem, SBUF receives fp16 for the DVE 2x-mode multiply).
  - Both full shards live in SBUF (8 MiB of 24), so no buffer recycling
    gates the DMA queues; compute chases the chunk-completion semaphores.
  - Per 2048-wide tile: VectorE fp16 in-place product (2x mode).  The
    free-dim reduction to fp32 partials is split: 6 tiles on the otherwise
    idle ScalarE (activation-Copy accum_out), 2 tiles on VectorE
    (tensor_scalar accum) so neither engine exceeds the DMA stream time.
    (The fused tensor_tensor_reduce would do mul+reduce in one op but this
    walrus build rejects its ISA encoding.)
  - The final all-reduce of the 8*128*NT fp32 partials is done on the host
    (a few KB), followed by the out-of-range predicate.
"""

import numpy as np

import concourse.bass as bass
import concourse.mybir as mybir
from concourse.bass_utils import run_bass_kernel_spmd

try:
    import ml_dtypes
    _F8 = np.dtype(ml_dtypes.float8_e4m3fn)
except ImportError:  # pragma: no cover
    _F8 = None

# Grid constants (must match the problem's reference.py)
N = 16777216
X0 = 0.0
DX = 1.0
XMAX = X0 + (N - 1) * DX

NCORES = 8
P = 128                 # SBUF partitions
S = N // NCORES         # 2,097,152 elements per core
F = S // P              # 16,384 free-dim elements per partition
T = 2048                # compute tile free width
NT = F // T             # 8 compute tiles per shard
CH = 4096               # DMA chunk free width (2 compute tiles)
NC = F // CH            # 4 chunks per ring
VEC_ACC = (2, 5)        # tiles reduced on VectorE; the rest on ScalarE

_CACHED_NC = None


def _build_nc():
    """Raw Bass (not Tile): this walrus build rejects instructions carrying
    more than ~1 inline semaphore wait ("Too many sync wait commands"), so
    all cross-engine sync uses standalone wait_ge instructions."""
    nc = bass.Bass(trn_type="TRN2")
    f16 = mybir.dt.float16
    f8 = mybir.dt.float8e4
    f32 = mybir.dt.float32
    yt = nc.dram_tensor("yt", [P, F], f16, kind="ExternalInput")
    mt = nc.dram_tensor("mt", [P, F], f8, kind="ExternalInput")
    out = nc.dram_tensor("out", [P, NT], f32, kind="ExternalOutput")

    with (
        nc.Block() as block,
        nc.semaphore("dy") as dy,
        nc.semaphore("dm") as dm,
        nc.semaphore("mul_sem") as mul_sem,
        nc.semaphore("acc_sem") as acc_sem,
        nc.semaphore("out_sem") as out_sem,
        nc.sbuf_tensor("ys", [P, F], f16) as ys,
        nc.sbuf_tensor("ms", [P, F], f16) as ms,
        nc.sbuf_tensor("acc", [P, NT], f32) as acc,
    ):
        @block.sync
        def _(sync):
            for k in range(NC):
                sync.dma_start(
                    out=ys[:, k * CH:(k + 1) * CH], in_=yt[:, k * CH:(k + 1) * CH]
                ).then_inc(dy, 16)
            sync.wait_ge(acc_sem, NT)
            sync.dma_start(out=out[:], in_=acc[:]).then_inc(out_sem, 16)
            sync.wait_ge(out_sem, 16)

        @block.gpsimd
        def _(gpsimd):
            for k in range(NC):
                # SWDGE casts fp8 -> fp16 inline during the transfer
                gpsimd.dma_start(
                    out=ms[:, k * CH:(k + 1) * CH], in_=mt[:, k * CH:(k + 1) * CH]
                ).then_inc(dm, 16)

        @block.vector
        def _(vector):
            for i in range(NT):
                k = (i * T) // CH
                vector.wait_ge(dy, 16 * (k + 1))
                vector.wait_ge(dm, 16 * (k + 1))
                sl = slice(i * T, (i + 1) * T)
                # in-place product into the y shard (fp16 -> DVE 2x mode)
                nc.vector.tensor_mul(
                    out=ys[:, sl], in0=ys[:, sl], in1=ms[:, sl]
                ).then_inc(mul_sem, 1)
                if i in VEC_ACC:
                    nc.vector.tensor_scalar(
                        ys[:, sl], ys[:, sl], 1.0, 0.0,
                        op0=mybir.AluOpType.mult, op1=mybir.AluOpType.add,
                        accum_out=acc[:, i:i + 1],
                    ).then_inc(acc_sem, 1)

        @block.scalar
        def _(scalar):
            for i in range(NT):
                if i in VEC_ACC:
                    continue
                scalar.wait_ge(mul_sem, i + 1)
                sl = slice(i * T, (i + 1) * T)
                # acc[:, i] = per-partition free-dim sum of the product;
                # the mandatory full-width copy lands in the (dead) mask tile
                nc.scalar.activation(
                    out=ms[:, sl],
                    in_=ys[:, sl],
                    func=mybir.ActivationFunctionType.Copy,
                    accum_out=acc[:, i:i + 1],
                ).then_inc(acc_sem, 1)

    return nc


def _get_nc():
    global _CACHED_NC
    if _CACHED_NC is None:
        _CACHED_NC = _build_nc()
    return _CACHED_NC


def kernel(x, yOrig, mask):
    x = np.asarray(x)
    yOrig = np.asarray(yOrig, dtype=np.float32)
    mask = np.asarray(mask, dtype=np.float32)

    xs = float(x.reshape(-1)[0])
    ind = int(np.floor((xs - X0) / DX))
    shift = ind % N

    # rolled[i] = mask[(i - ind) mod N]  (== np.roll(mask, ind))
    if shift == 0:
        rolled = mask
    else:
        rolled = np.concatenate([mask[N - shift:], mask[:N - shift]])

    yq = yOrig.astype(np.float16)
    mq = rolled.astype(_F8).view(np.uint8)

    in_maps = []
    for c in range(NCORES):
        in_maps.append({
            "yt": yq[c * S:(c + 1) * S].reshape(P, F),
            "mt": mq[c * S:(c + 1) * S].reshape(P, F),
        })

    res = run_bass_kernel_spmd(_get_nc(), in_maps, core_ids=list(range(NCORES)))

    partials = np.concatenate([r["out"].reshape(-1) for r in res.results])
    total = np.float32(partials.sum(dtype=np.float32))

    if xs >= XMAX or xs < X0:
        total = np.float32(0.0)

    # Stash for test harnesses that want profiling info.
    kernel.last_results = res
    return np.asarray(total, dtype=np.float32)


# revision 7
# speedup vs baseline: 1.0089x; 1.0089x over previous
"""Trainium2 kernel for nn_InterpolatorMaskArgs (embedding_lookup, memory regime).

reference computes:  ind = floor((x[0]-X0)/DX);  res = sum(roll(mask, ind) * yOrig)
i.e. a full O(N) dot product between yOrig and the rolled mask, with an
out-of-range guard on x.

Strategy (matches the sharding hint):
  - 1-D shard yOrig along N across the 8 cores (contiguous 2M-element shards).
  - The roll is resolved at shard time: core c receives the slice of the
    rolled mask aligned with its yOrig shard, i.e. mask[(c*S - ind) mod N ...]
    (mod-N wraparound == the halo exchange, done while scattering inputs).
  - Both inputs are cast to fp16 on the host (tolerance is 2e-2; measured
    end-to-end rel err ~1e-3).  8 MiB/core of DMA; the binding resource is
    the SDMA SBUF-write side at ~340 GB/s aggregate (fp8 variants measured
    here don't help: cast-DMA still writes fp16, and fp8-consuming DVE ops
    run 4-5x slower than fp16 2x mode).
  - THREE DMA queues run concurrently so per-transfer completion overheads
    overlap: y chunks on the sync HWDGE ring, the two small tapered tail
    chunks of y on the scalar HWDGE ring, mask chunks on the gpsimd SWDGE
    ring.
  - Both full shards live in SBUF (8 MiB of 24) so no buffer recycling gates
    the queues; compute chases the per-chunk completion semaphores, and the
    final chunks/tiles are tapered (1024 wide) to shrink the serial tail.
  - Per tile: VectorE fp16 in-place product (DVE 2x mode).  The free-dim
    reduction to fp32 partials is split: most tiles on the otherwise idle
    ScalarE (activation-Copy accum_out), two mid-stream tiles on VectorE
    (tensor_scalar accum) so neither engine exceeds the DMA stream time.
    (The fused tensor_tensor_reduce would do mul+reduce in one op but this
    walrus build rejects its ISA encoding.)
  - The final all-reduce of the 8*128*NTILES fp32 partials is done on the
    host (a few KB), followed by the out-of-range predicate.
"""

import numpy as np

import concourse.bass as bass
import concourse.mybir as mybir
from concourse.bass_utils import run_bass_kernel_spmd

# Grid constants (must match the problem's reference.py)
N = 16777216
X0 = 0.0
DX = 1.0
XMAX = X0 + (N - 1) * DX

NCORES = 8
P = 128                 # SBUF partitions
S = N // NCORES         # 2,097,152 elements per core
F = S // P              # 16,384 free-dim elements per partition

# y chunks: 4 on the sync ring, then 2 tapered on the scalar ring
Y_SYNC = [(0, 4096), (4096, 8192), (8192, 12288), (12288, 14336)]
Y_SCAL = [(14336, 15360), (15360, 16384)]
# mask chunks (gpsimd ring) match the union, in order
M_CHUNKS = Y_SYNC + Y_SCAL
# compute tiles: (start, end, y-ring ('s'=sync,'a'=scalar), y-chunk#, m-chunk#)
TILES = [
    (0, 2048, 's', 1, 1),
    (2048, 4096, 's', 1, 1),
    (4096, 6144, 's', 2, 2),
    (6144, 8192, 's', 2, 2),
    (8192, 10240, 's', 3, 3),
    (10240, 12288, 's', 3, 3),
    (12288, 14336, 's', 4, 4),
    (14336, 15360, 'a', 1, 5),
    (15360, 16384, 'a', 2, 6),
]
NTILES = len(TILES)
VEC_ACC = (2, 5)        # tiles reduced on VectorE; the rest on ScalarE

_CACHED_NC = None


def _build_nc():
    """Raw Bass (not Tile): this walrus build rejects instructions carrying
    more than ~1 inline semaphore wait ("Too many sync wait commands"), so
    all cross-engine sync uses standalone wait_ge instructions."""
    nc = bass.Bass(trn_type="TRN2")
    f16 = mybir.dt.float16
    f32 = mybir.dt.float32
    yt = nc.dram_tensor("yt", [P, F], f16, kind="ExternalInput")
    mt = nc.dram_tensor("mt", [P, F], f16, kind="ExternalInput")
    out = nc.dram_tensor("out", [P, NTILES], f32, kind="ExternalOutput")

    with (
        nc.Block() as block,
        nc.semaphore("dy") as dy,
        nc.semaphore("db") as db,
        nc.semaphore("dm") as dm,
        nc.semaphore("mul_sem") as mul_sem,
        nc.semaphore("acc_sem") as acc_sem,
        nc.semaphore("out_sem") as out_sem,
        nc.sbuf_tensor("ys", [P, F], f16) as ys,
        nc.sbuf_tensor("ms", [P, F], f16) as ms,
        nc.sbuf_tensor("acc", [P, NTILES], f32) as acc,
    ):
        @block.sync
        def _(sync):
            for a, b in Y_SYNC:
                sync.dma_start(out=ys[:, a:b], in_=yt[:, a:b]).then_inc(dy, 16)
            sync.wait_ge(acc_sem, NTILES)
            sync.dma_start(out=out[:], in_=acc[:]).then_inc(out_sem, 16)
            sync.wait_ge(out_sem, 16)

        @block.gpsimd
        def _(gpsimd):
            for a, b in M_CHUNKS:
                gpsimd.dma_start(out=ms[:, a:b], in_=mt[:, a:b]).then_inc(dm, 16)

        @block.vector
        def _(vector):
            for i, (a, b, ring, yk, mk) in enumerate(TILES):
                vector.wait_ge(dy if ring == 's' else db, 16 * yk)
                vector.wait_ge(dm, 16 * mk)
                # in-place product into the y shard (fp16 -> DVE 2x mode)
                nc.vector.tensor_mul(
                    out=ys[:, a:b], in0=ys[:, a:b], in1=ms[:, a:b]
                ).then_inc(mul_sem, 1)
                if i in VEC_ACC:
                    nc.vector.tensor_scalar(
                        ys[:, a:b], ys[:, a:b], 1.0, 0.0,
                        op0=mybir.AluOpType.mult, op1=mybir.AluOpType.add,
                        accum_out=acc[:, i:i + 1],
                    ).then_inc(acc_sem, 1)

        @block.scalar
        def _(scalar):
            # issue the two tapered tail y-chunks up front on the ACT ring
            for a, b in Y_SCAL:
                scalar.dma_start(out=ys[:, a:b], in_=yt[:, a:b]).then_inc(db, 16)
            for i, (a, b, ring, yk, mk) in enumerate(TILES):
                if i in VEC_ACC:
                    continue
                scalar.wait_ge(mul_sem, i + 1)
                # acc[:, i] = per-partition free-dim sum of the product;
                # the mandatory full-width copy lands in the (dead) mask tile
                nc.scalar.activation(
                    out=ms[:, a:b],
                    in_=ys[:, a:b],
                    func=mybir.ActivationFunctionType.Copy,
                    accum_out=acc[:, i:i + 1],
                ).then_inc(acc_sem, 1)

    return nc


def _get_nc():
    global _CACHED_NC
    if _CACHED_NC is None:
        _CACHED_NC = _build_nc()
    return _CACHED_NC


def kernel(x, yOrig, mask):
    x = np.asarray(x)
    yOrig = np.asarray(yOrig, dtype=np.float32)
    mask = np.asarray(mask, dtype=np.float32)

    xs = float(x.reshape(-1)[0])
    ind = int(np.floor((xs - X0) / DX))
    shift = ind % N

    # rolled[i] = mask[(i - ind) mod N]  (== np.roll(mask, ind))
    if shift == 0:
        rolled = mask
    else:
        rolled = np.concatenate([mask[N - shift:], mask[:N - shift]])

    yq = yOrig.astype(np.float16)
    mq = rolled.astype(np.float16)

    in_maps = []
    for c in range(NCORES):
        in_maps.append({
            "yt": yq[c * S:(c + 1) * S].reshape(P, F),
            "mt": mq[c * S:(c + 1) * S].reshape(P, F),
        })

    res = run_bass_kernel_spmd(_get_nc(), in_maps, core_ids=list(range(NCORES)))

    partials = np.concatenate([r["out"].reshape(-1) for r in res.results])
    total = np.float32(partials.sum(dtype=np.float32))

    if xs >= XMAX or xs < X0:
        total = np.float32(0.0)

    # Stash for test harnesses that want profiling info.
    kernel.last_results = res
    return np.asarray(total, dtype=np.float32)
